# revision 10
# baseline (speedup 1.0000x reference)
"""BlockAttentionResidual Trainium2 kernel.

Math (per (b,t) row, V slice v_n of length D, n = 0..7):
    ssq_n = sum(v_n^2)
    rms_n = rsqrt(ssq_n / D + eps)
    logit_n = rms_n * dot(v_n, qw)        with qw = key_norm_weight * pseudo_query
    w = softmax(logit)                     over n
    out = sum_n w_n * v_n

Sharding: rows (B*T flattened) split evenly across 8 cores; (D,) params
replicated. No cross-core communication.

Per-core layout: tiles of 32 rows; SBUF tile (128, 2D) with partition
p = 8*r + n holding HBM-contiguous rows 2r and 2r+1 of plane n (16 KiB
per partition line -> efficient DMA).
  - ssq: ScalarE activation(Square, accum_out), one call per row-half
  - dot: VectorE scalar_tensor_tensor(mult, accum_out), one per row-half
  - softmax over n: PE-transpose the (128, 8) scalar columns so n lands
    innermost on the free axis, then max/exp/sum/div, transpose back
  - weighted sum: PE matmul; stationary (128, 32) block-diagonals place
    even rows at m=r and odd rows at m=16+r, two accumulating matmuls
    per 512-chunk; 4 tiles col-tiled into one full (128, D) PSUM region,
    ScalarE copy to SBUF, single 1 MiB store per 128 rows.
DMA rings: input tiles on the SP HWDGE ring, output stores on the ACT
HWDGE ring.
"""

import os
import sys

for _p in ("/opt/trn_rl_repo",):
    if _p not in sys.path and os.path.isdir(_p):
        sys.path.append(_p)

import numpy as np

import concourse.bass as bass
import concourse.tile as tile
from concourse import bacc, mybir
from concourse.bass_utils import run_bass_kernel_spmd

N_CORES = 8
N = 8          # depth entries (softmax axis)
B = 4
T = 2048
D = 2048
R_TOTAL = B * T            # 8192 rows
RPC = R_TOTAL // N_CORES   # 1024 rows per core
TR = 32                    # rows per tile (16 r-slots x 2 rows x 8 n)
EPS = 1e-6
NCHUNK = 512               # matmul moving free-dim chunk (fp32 max)

F32 = mybir.dt.float32
ALU = mybir.AluOpType
ACTF = mybir.ActivationFunctionType


def build_program(rows_per_core=RPC, debug=False, xbufs=6):
    """Build the per-core Bass program (identical on all cores)."""
    nt = rows_per_core // TR           # tiles per core
    nc = bacc.Bacc(
        "TRN2", target_bir_lowering=False, debug=debug, num_devices=N_CORES
    )

    v_dram = nc.dram_tensor("V", (N, rows_per_core, D), F32, kind="ExternalInput").ap()
    qw_dram = nc.dram_tensor("QW", (128, D), F32, kind="ExternalInput").ap()
    ee_dram = nc.dram_tensor("EE", (128, 32), F32, kind="ExternalInput").ap()
    eo_dram = nc.dram_tensor("EO", (128, 32), F32, kind="ExternalInput").ap()
    id_dram = nc.dram_tensor("ID", (128, 128), F32, kind="ExternalInput").ap()
    out_dram = nc.dram_tensor(
        "OUT", (rows_per_core, D), F32, kind="ExternalOutput"
    ).ap()

    with tile.TileContext(nc) as tc:
        with (
            tc.tile_pool(name="consts", bufs=1) as consts,
            tc.tile_pool(name="xpool", bufs=xbufs) as xpool,
            tc.tile_pool(name="scratch", bufs=2) as scratch,
            tc.tile_pool(name="outpool", bufs=2) as outpool,
            tc.tile_pool(name="smalls", bufs=3) as smalls,
            tc.tile_pool(name="wdpool", bufs=6) as wdpool,
            tc.tile_pool(name="psum_big", bufs=1, space="PSUM") as psum_big_pool,
            tc.tile_pool(name="psum_sm", bufs=2, space="PSUM") as psum_sm,
        ):
            qw_sb = consts.tile([128, D], F32)
            nc.sync.dma_start(qw_sb[:], qw_dram[:])
            ee_sb = consts.tile([128, 32], F32)
            nc.sync.dma_start(ee_sb[:], ee_dram[:])
            eo_sb = consts.tile([128, 32], F32)
            nc.sync.dma_start(eo_sb[:], eo_dram[:])
            id_sb = consts.tile([128, 128], F32)
            nc.sync.dma_start(id_sb[:], id_dram[:])
            zero_sb = consts.tile([128, 1], F32)
            nc.vector.memset(zero_sb[:], 0.0)
            eps_sb = consts.tile([128, 1], F32)
            nc.vector.memset(eps_sb[:], EPS)

            assert nt % 4 == 0, "tiles per core must be a multiple of 4"
            for g in range(nt // 4):          # groups of 4 tiles = 128 rows
                xt = []
                dots = smalls.tile([128, 8], F32, tag="dots")
                ssqs = smalls.tile([128, 8], F32, tag="ssqs")
                for j in range(4):
                    t = 4 * g + j
                    x = xpool.tile([128, 2 * D], F32, tag="x")
                    # partition p = 8r+n <- rows (32t+2r, 32t+2r+1) of plane n
                    src = (
                        v_dram[:, TR * t : TR * (t + 1), :]
                        .rearrange("n (r two) d -> r n (two d)", two=2)
                    )
                    nc.sync.dma_start(x[:], src)
                    xt.append(x)

                    for eo in range(2):
                        xh = x[:, D * eo : D * (eo + 1)]
                        prod = scratch.tile([128, D], F32, tag="prod")
                        nc.vector.scalar_tensor_tensor(
                            out=prod[:],
                            in0=xh,
                            scalar=1.0,
                            in1=qw_sb[:],
                            op0=ALU.mult,
                            op1=ALU.mult,
                            accum_out=dots[:, 2 * j + eo : 2 * j + eo + 1],
                        )
                        sq = scratch.tile([128, D], F32, tag="sq")
                        nc.scalar.activation(
                            sq[:], xh, ACTF.Square, bias=zero_sb[:],
                            accum_out=ssqs[:, 2 * j + eo : 2 * j + eo + 1],
                        )

                # logits = dot * rsqrt(ssq/D + eps)  — (128, 8)
                snorm = smalls.tile([128, 8], F32, tag="snorm")
                nc.scalar.activation(
                    snorm[:], ssqs[:], ACTF.Sqrt, bias=eps_sb[:], scale=1.0 / D
                )
                rms = smalls.tile([128, 8], F32, tag="rms")
                nc.vector.reciprocal(rms[:], snorm[:])
                logits = smalls.tile([128, 8], F32, tag="logits")
                nc.vector.tensor_mul(logits[:], dots[:], rms[:])

                # transpose to (8, 128) so n is innermost on the free axis
                ps_t = psum_sm.tile([8, 128], F32, tag="pst")
                nc.tensor.transpose(ps_t[:], logits[:], id_sb[:])
                tsb = smalls.tile([8, 128], F32, tag="tsb")
                nc.scalar.copy(tsb[:], ps_t[:])
                t3 = tsb[:].rearrange("p (r n) -> p r n", n=N)

                negmax = smalls.tile([8, 16], F32, tag="negmax")
                nc.vector.tensor_reduce(
                    negmax[:], t3, axis=mybir.AxisListType.X, op=ALU.max, negate=True
                )
                shifted = smalls.tile([8, 128], F32, tag="shifted")
                sh3 = shifted[:].rearrange("p (r n) -> p r n", n=N)
                nmb = negmax[:].unsqueeze(2).broadcast_to([8, 16, N])
                nc.vector.tensor_tensor(sh3, t3, nmb, ALU.add)
                expd = smalls.tile([8, 128], F32, tag="expd")
                nc.scalar.activation(expd[:], shifted[:], ACTF.Exp, bias=zero_sb[0:8])
                ex3 = expd[:].rearrange("p (r n) -> p r n", n=N)
                sums = smalls.tile([8, 16], F32, tag="sums")
                nc.vector.tensor_reduce(
                    sums[:], ex3, axis=mybir.AxisListType.X, op=ALU.add
                )
                rsums = smalls.tile([8, 16], F32, tag="rsums")
                nc.vector.reciprocal(rsums[:], sums[:])
                wts = smalls.tile([8, 128], F32, tag="wts")
                w3 = wts[:].rearrange("p (r n) -> p r n", n=N)
                rsb = rsums[:].unsqueeze(2).broadcast_to([8, 16, N])
                nc.vector.tensor_tensor(w3, ex3, rsb, ALU.mult)

                # transpose back: column 2j+eo = weights for (tile j, parity eo)
                ps_w = psum_sm.tile([128, 8], F32, tag="psw")
                nc.tensor.transpose(ps_w[:], wts[:], id_sb[0:8, 0:8])
                wcols = smalls.tile([128, 8], F32, tag="wcols")
                nc.scalar.copy(wcols[:], ps_w[:])

                # weighted sum via PE; 4 tiles col-tiled into one PSUM region
                psb = psum_big_pool.tile([128, D], F32, tag="psb")
                for c in range(4):
                    we = wdpool.tile([128, 32], F32, tag="we")
                    nc.vector.tensor_scalar(
                        out=we[:], in0=ee_sb[:], scalar1=wcols[:, 2 * c : 2 * c + 1],
                        scalar2=None, op0=ALU.mult,
                    )
                    wo = wdpool.tile([128, 32], F32, tag="wo")
                    nc.vector.tensor_scalar(
                        out=wo[:], in0=eo_sb[:],
                        scalar1=wcols[:, 2 * c + 1 : 2 * c + 2],
                        scalar2=None, op0=ALU.mult,
                    )
                    for k in range(D // NCHUNK):
                        ps_slice = psb[
                            32 * c : 32 * (c + 1), NCHUNK * k : NCHUNK * (k + 1)
                        ]
                        nc.tensor.matmul(
                            ps_slice, we[:],
                            xt[c][:, NCHUNK * k : NCHUNK * (k + 1)],
                            start=True, stop=False, tile_position=(0, 32 * c),
                        )
                        nc.tensor.matmul(
                            ps_slice, wo[:],
                            xt[c][:, D + NCHUNK * k : D + NCHUNK * (k + 1)],
                            start=False, stop=True, tile_position=(0, 32 * c),
                        )
                osb = outpool.tile([128, D], F32, tag="osb")
                nc.scalar.copy(osb[:], psb[:])
                # masks put row 32c+2r+eo at partition 32c+2r+eo: plain store
                nc.scalar.dma_start(out_dram[128 * g : 128 * (g + 1), :], osb[:])

    nc.compile()
    return nc


def make_consts():
    """Host-side constants: even/odd block-diagonal masks and identity."""
    ee = np.zeros((128, 32), dtype=np.float32)
    eo = np.zeros((128, 32), dtype=np.float32)
    for p in range(128):
        r = p // N
        ee[p, 2 * r] = 1.0
        eo[p, 2 * r + 1] = 1.0
    ident = np.eye(128, dtype=np.float32)
    return ee, eo, ident


def prepare_in_maps(V, key_norm_weight, pseudo_query, rows_per_core=RPC,
                    n_cores=N_CORES):
    qw = (np.asarray(key_norm_weight, dtype=np.float32)
          * np.asarray(pseudo_query, dtype=np.float32))
    qw_b = np.ascontiguousarray(np.broadcast_to(qw, (128, D)))
    ee, eo, ident = make_consts()
    vf = np.ascontiguousarray(np.asarray(V, dtype=np.float32)).reshape(N, -1, D)
    in_maps = []
    for c in range(n_cores):
        sl = np.ascontiguousarray(
            vf[:, c * rows_per_core : (c + 1) * rows_per_core, :]
        )
        in_maps.append({"V": sl, "QW": qw_b, "EE": ee, "EO": eo, "ID": ident})
    return in_maps


_PROGRAM_CACHE = {}


def _get_program():
    key = (RPC,)
    if key not in _PROGRAM_CACHE:
        _PROGRAM_CACHE[key] = build_program(RPC, debug=False)
    return _PROGRAM_CACHE[key]


def run(V, key_norm_weight, pseudo_query, trace=False, **trace_kwargs):
    nc = _get_program()
    in_maps = prepare_in_maps(V, key_norm_weight, pseudo_query)
    res = run_bass_kernel_spmd(
        nc, in_maps, list(range(N_CORES)), trace=trace, **trace_kwargs
    )
    out = np.empty((R_TOTAL, D), dtype=np.float32)
    for c in range(N_CORES):
        out[c * RPC : (c + 1) * RPC, :] = res.results[c]["OUT"]
    return out.reshape(B, T, D), res


def kernel(V, key_norm_weight, pseudo_query):
    out, _ = run(V, key_norm_weight, pseudo_query, trace=False)
    return out
